# revision 31
# baseline (speedup 1.0000x reference)
"""Trainium2 Bass kernel for nn_LocalAttn: grouped local attention (3x3 window).

Sharding: 8 cores = batch(2) x H-strips(4). Each core gets a 34-row slice
(32 output rows + 1 halo row each side) of the W-and-H zero-padded input,
so all cores run one identical SPMD program.

v2 design (channel-major, pixels on the free dim, W padded to 130):
  conv1 (PE, f32r block-diag) -> BN1+bias+tanh (ScalarE) -> t, with pad
  columns/halo rows of t zeroed so that conv2 can be FUSED into a direct
  3x3 logits conv on the PE: logits[8k+g] = w2m'.t(center) + w2n'.t(+dk),
  PSUM-accumulated over the 9 offsets (BN2 scales folded into weights).
  exp via ScalarE (bias = folded BN2 offset) -> e72 bf16. Softmax denom
  via ones-matmul (PE) -> reciprocal (DVE) -> rb f32.
  e72/rb round-trip through DRAM so cheap broadcast DMAs (stride-0 source
  dims) can expand 8 group rows -> 128 channel rows without touching any
  compute engine. Apply: per (quad, half): 9 broadcast DMAs feed bf16
  multiplies (DVE 2x mode + Pool scalar_tensor_tensor split), bf16 add
  tree on DVE, final 1/denom scale on Pool -> f32 out -> DMA out.
"""

import numpy as np
import ml_dtypes

import concourse.bass as bass
import concourse.bacc as bacc
import concourse.mybir as mybir
from concourse import tile
from concourse.bass_utils import run_bass_kernel_spmd

F32 = mybir.dt.float32
F32R = mybir.dt.float32r
BF16 = mybir.dt.bfloat16
AF = mybir.ActivationFunctionType
ALU = mybir.AluOpType

EPS = 1e-5
G = 8          # groups
B = 2
C = 256
H = W = 128
HS = 32        # output rows per core
HI = 34        # input rows per core (with halo)
WP = 130       # padded width
NIN = HI * WP          # 4420
NOUT = HS * WP         # 4160
NPAD = NIN + 2         # t / v free size, data at base offset 1

# phase A/C row-chunking (PSUM free dim <= 512 f32)
ACH = 3                # rows per conv chunk
FD = NOUT // 5         # apply tile: fifth of the output (832)
FDH = FD // 2          # PSUM-tile grain (416)

# free-dim offset into a base-1 padded [.., NPAD] tensor for the (dy,dx)
# neighbor of output pixel 0 (= input row 1, col 0)
def _koff(k):
    dy, dx = k // 3 - 1, k % 3 - 1
    return 1 + WP + dy * WP + dx


# apply-phase mul ownership: which k's multiply on DVE vs Pool
KS_DVE = (0, 1, 2, 3, 4, 5)   # bf16 SBUF muls on DVE
KS_POOLS = (6, 7, 8)          # bf16 SBUF muls on Pool (tensor_tensor)

_NC_CACHE = {}


def _build_nc():
    nc = bacc.Bacc("TRN2", target_bir_lowering=False, debug=False, num_devices=8)

    x_d = nc.dram_tensor("x", [2, 128, NIN], BF16, kind="ExternalInput")
    ca_d = nc.dram_tensor("cpkA", [128, 320], BF16, kind="ExternalInput")
    cs_d = nc.dram_tensor("cpkS", [64, 8], F32, kind="ExternalInput")
    cf2_d = nc.dram_tensor("cpkF", [72, 721], F32R, kind="ExternalInput")
    cb_d = nc.dram_tensor("cpkB", [72, 80], BF16, kind="ExternalInput")
    out_d = nc.dram_tensor("out", [2, 128, NOUT], F32, kind="ExternalOutput")
    # DRAM scratch for broadcast staging
    e72_d = nc.dram_tensor("e72d", [72, NOUT], BF16, kind="Internal")

    nch = HI // ACH + (1 if HI % ACH else 0)       # 12 input chunks (11x3+1)
    ncho = HS // ACH + (1 if HS % ACH else 0)      # 11 output chunks (10x3+2)

    with tile.TileContext(nc) as tc:
        with (
            nc.allow_low_precision(reason="bf16 softmax weights/values"),
            tc.tile_pool(name="const", bufs=1) as cp,
            tc.tile_pool(name="mid", bufs=1) as mp,
        ):
            # ---- input loads first (SP/ACT), then weights ----
            xq = []
            for q in range(2):
                xt = mp.tile([128, NIN], BF16, tag=f"x_{q}", name=f"x_{q}")
                xq.append(xt)
            xrows = [(0, 4), (4, 9), (9, 16), (16, 25), (25, 34)]
            for ci, (ra, rb_) in enumerate(xrows):
                fsl = slice(ra * WP, rb_ * WP)
                for q in range(2):
                    eng = nc.sync if (ci + q) % 2 == 0 else nc.scalar
                    eng.dma_start(xq[q][:, fsl], x_d[q, :, fsl])

            # ---- constant / weight loads (packed, few DMAs) ----
            cA = cp.tile([128, 320], BF16, tag="cA", name="cA")
            nc.gpsimd.dma_start(cA[:], ca_d[:])
            w1t = [cA[:, 0:32], cA[:, 32:64]]
            wvt = [cA[:, 64:192], cA[:, 192:320]]
            cS = cp.tile([64, 8], F32, tag="cS", name="cS")
            nc.gpsimd.dma_start(cS[:], cs_d[:])
            s1t = cS[:, 0:1]
            c1t = cS[:, 1:2]
            atop = cS[0:8, 2:3]
            btop = cS[0:8, 3:4]
            abot = cS[0:8, 4:5]
            bbot = cS[0:8, 5:6]
            ubc = cS[0:8, 6:7]
            cF = cp.tile([72, 721], F32R, tag="cF", name="cF")
            nc.sync.dma_start(cF[:], cf2_d[:])
            w2mt = cF[0:64, 0:72]
            w2nt = [cF[0:8, 72 + 72 * k : 144 + 72 * k] for k in range(9)]
            cft = cF[0:72, 720:721]
            cB = cp.tile([72, 80], BF16, tag="cB", name="cB")
            nc.sync.dma_start(cB[:], cb_d[:])
            onest = cB[0:72, 0:8]
            r72t = cB[0:8, 8:80]

            # ---- persistent mid tensors ----
            t_sb = mp.tile([64, NPAD], F32R, tag="t", name="t")
            v_sb = [mp.tile([128, NPAD], BF16, tag=f"v_{q}", name=f"v_{q}") for q in range(2)]
            e72 = mp.tile([72, NOUT], BF16, tag="e72", name="e72")
            rb = mp.tile([8, NOUT], BF16, tag="rb", name="rb")

            # ---- phases A+C interleaved: conv1/tanh/vconv then fused
            # conv2/exp/denom two chunks behind, sharing one PSUM scope ----
            for q in range(2):
                nc.gpsimd.memset(v_sb[q][:, 0:1], 0.0)
                nc.gpsimd.memset(v_sb[q][:, NPAD - 1 : NPAD], 0.0)
            with (
                tc.tile_pool(name="pc64", bufs=2, space="PSUM") as pc64,
                tc.tile_pool(name="pv", bufs=2, space="PSUM") as pvp,
                tc.tile_pool(name="pcm", bufs=2, space="PSUM") as pcm,
                tc.tile_pool(name="pcd", bufs=1, space="PSUM") as pcd,
                tc.tile_pool(name="pcr", bufs=1, space="PSUM") as pcr,
            ):
                def a_chunk(ch):
                    r0 = ch * ACH
                    r1 = min(r0 + ACH, HI)
                    f0 = r0 * WP
                    fsz = (r1 - r0) * WP
                    sl = slice(f0, f0 + fsz)
                    pt = pc64.tile([64, fsz], F32, tag="pc", name="pc", padded_shape=[64, 512])
                    nc.tensor.matmul(
                        pt[0:32, :], w1t[0],
                        xq[0][:, sl],
                        start=True, stop=True, tile_position=(0, 0),
                    )
                    nc.tensor.matmul(
                        pt[32:64, :], w1t[1],
                        xq[1][:, sl],
                        start=True, stop=True, tile_position=(0, 32),
                    )
                    nc.scalar.activation(
                        t_sb[:, 1 + f0 : 1 + f0 + fsz], pt[:],
                        AF.Tanh, bias=c1t, scale=s1t,
                    )
                    # boundary cells of t (group-0 rows): halo rows become
                    # t*a + b (a,b host-set: u at image boundary, identity
                    # elsewhere); W-pad columns always become u, so the fused
                    # neighbor term cancels the folded BN bias off-image
                    if ch == 0:
                        nc.vector.tensor_scalar(
                            t_sb[0:8, 1 : 1 + WP], t_sb[0:8, 1 : 1 + WP],
                            atop, btop, ALU.mult, ALU.add,
                        )
                    if ch == nch - 1:
                        nc.vector.tensor_scalar(
                            t_sb[0:8, 1 + 33 * WP : 1 + 34 * WP],
                            t_sb[0:8, 1 + 33 * WP : 1 + 34 * WP],
                            abot, bbot, ALU.mult, ALU.add,
                        )
                    nc.vector.tensor_scalar(
                        t_sb[0:8, f0 : f0 + fsz : WP],
                        xq[0][0:8, 0 : fsz : WP],
                        0.0, ubc, ALU.mult, ALU.add,
                    )
                    nc.vector.tensor_scalar(
                        t_sb[0:8, 1 + f0 : 1 + f0 + fsz : WP],
                        xq[0][0:8, 0 : fsz : WP],
                        0.0, ubc, ALU.mult, ALU.add,
                    )
                    if ch == nch - 1:
                        nc.vector.tensor_scalar(
                            t_sb[0:8, f0 + fsz : NPAD],
                            xq[0][0:8, 0 : NPAD - f0 - fsz],
                            0.0, ubc, ALU.mult, ALU.add,
                        )
                def v_chunk(ch):
                    r0 = ch * ACH
                    r1 = min(r0 + ACH, HI)
                    f0 = r0 * WP
                    fsz = (r1 - r0) * WP
                    sl = slice(f0, f0 + fsz)
                    for q in range(2):
                        pv = pvp.tile([128, fsz], F32, tag="pv", name="pv", padded_shape=[128, 512])
                        nc.tensor.matmul(
                            pv[:], wvt[q],
                            xq[q][:, sl],
                            start=True, stop=True,
                        )
                        nc.scalar.copy(v_sb[q][:, 1 + f0 : 1 + f0 + fsz], pv[:])

                def c_chunk(ch):
                    r0 = ch * ACH
                    r1 = min(r0 + ACH, HS)
                    fsz = (r1 - r0) * WP
                    o0 = r0 * WP                     # offset in out space
                    tbase = 1 + WP + o0              # center in t space
                    pm = pcm.tile([72, fsz], F32, tag="pm", name="pm", padded_shape=[72, 512])
                    nc.tensor.matmul(
                        pm[:], w2mt,
                        t_sb[:, tbase : tbase + fsz],
                        start=True, stop=False, skip_group_check=True,
                    )
                    for k in range(9):
                        dy, dx = k // 3 - 1, k % 3 - 1
                        tb = tbase + dy * WP + dx
                        nc.tensor.matmul(
                            pm[:],
                            w2nt[k],
                            t_sb[0:8, tb : tb + fsz],
                            start=False, stop=(k == 8), skip_group_check=True,
                        )
                    nc.scalar.activation(
                        e72[:, o0 : o0 + fsz], pm[:],
                        AF.Exp, bias=cft,
                    )
                    pd = pcd.tile([8, fsz], F32, tag="pd", name="pd", padded_shape=[8, 512])
                    nc.tensor.matmul(
                        pd[:], onest, e72[:, o0 : o0 + fsz],
                        start=True, stop=True,
                    )
                    nc.vector.reciprocal(rb[:, o0 : o0 + fsz], pd[:])
                    pr72 = pcr.tile([72, fsz], F32, tag="pr72", name="pr72", padded_shape=[72, 512])
                    nc.tensor.matmul(
                        pr72[:], r72t, rb[:, o0 : o0 + fsz],
                        start=True, stop=True,
                    )
                    nc.vector.tensor_mul(
                        e72[:, o0 : o0 + fsz], e72[:, o0 : o0 + fsz], pr72[:]
                    )

                LAG = 2
                for ch in range(nch + ncho):
                    if ch < nch:
                        a_chunk(ch)
                    if LAG <= ch and ch - LAG < ncho:
                        c_chunk(ch - LAG)
                    if LAG <= ch and ch - LAG < nch:
                        v_chunk(ch - LAG)
                for ch in range(nch - LAG, nch):
                    v_chunk(ch)

            # stage normalized e72 to DRAM per fifth (for the pab broadcasts)
            for h in range(5):
                hsl = slice(h * FD, (h + 1) * FD)
                nc.scalar.dma_start(e72_d[:, hsl], e72[:, hsl])

            # ---- phase D: apply ----
            with (
                nc.allow_low_precision(reason="3x3 softmax-weighted sum in bf16"),
                tc.tile_pool(name="pab", bufs=18) as pabp,
                tc.tile_pool(name="prod", bufs=11) as prp,
                tc.tile_pool(name="sums", bufs=6) as smp,
                tc.tile_pool(name="outp", bufs=3) as outp,
            ):
                for h in range(5):
                    for q in range(2):
                        h0 = h * FD
                        hsl = slice(h0, h0 + FD)
                        pab = {}
                        for k in range(9):
                            pt = pabp.tile([128, FD], BF16, tag="pab", name="pab")
                            src_ap = (
                                e72_d[8 * k + 4 * q : 8 * k + 4 * q + 4, hsl]
                                .unsqueeze(1).broadcast_to([4, 32, FD])
                            )
                            eng = nc.sync if k in (0, 2, 4, 6, 7, 8) else nc.scalar
                            eng.dma_start(pt[:], src_ap)
                            pab[k] = pt
                        prod = {}
                        for k in range(9):
                            voff = _koff(k) + h0
                            vsl = v_sb[q][:, voff : voff + FD]
                            pr = prp.tile([128, FD], BF16, tag="pr", name="pr")
                            if k in KS_DVE:
                                nc.vector.tensor_mul(pr[:], pab[k][:], vsl)
                            else:
                                nc.gpsimd.tensor_mul(pr[:], pab[k][:], vsl)
                            prod[k] = pr
                        # balanced bf16 add tree on DVE; final add on Pool
                        s01 = smp.tile([128, FD], BF16, tag="s", name="s01")
                        nc.vector.tensor_add(s01[:], prod[0][:], prod[1][:])
                        s23 = smp.tile([128, FD], BF16, tag="s", name="s23")
                        nc.vector.tensor_add(s23[:], prod[2][:], prod[3][:])
                        s45 = smp.tile([128, FD], BF16, tag="s", name="s45")
                        nc.vector.tensor_add(s45[:], prod[4][:], prod[5][:])
                        nc.vector.tensor_add(s01[:], s01[:], s23[:])
                        nc.vector.tensor_add(s01[:], s01[:], s45[:])
                        nc.vector.tensor_add(s01[:], s01[:], prod[6][:])
                        nc.vector.tensor_add(s01[:], s01[:], prod[7][:])
                        ot = outp.tile([128, FD], F32, tag="ot", name="ot")
                        nc.gpsimd.tensor_add(ot[:], s01[:], prod[8][:])
                        eng_o = nc.sync if (h + q) % 2 == 0 else nc.scalar
                        eng_o.dma_start(out_d[q, :, hsl], ot[:])

    nc.compile()
    return nc


def _host_prep(x, w1, b1, g1, be1, m1, v1, w2, b2, g2, be2, m2, v2, wv):
    f32 = np.float32

    inv1 = (g1 / np.sqrt(v1 + EPS)).astype(f32)            # [64]
    s1 = inv1
    c1 = (b1 * inv1 + be1 - m1 * inv1).astype(f32)
    inv2 = (g2 / np.sqrt(v2 + EPS)).astype(f32)            # [80]
    s2r = inv2
    c2r = (b2 * inv2 + be2 - m2 * inv2).astype(f32)

    # conv2 output layout: psum row j = 8k+g -> ref mask ch 8+9g+k;
    # neighbor path: ref ch g (g<8), i.e. group 0 of t, co=g
    mperm = np.zeros(72, dtype=np.int64)
    for k in range(9):
        for g in range(8):
            mperm[8 * k + g] = 8 + 9 * g + k
    s2m = s2r[mperm]
    c2m = c2r[mperm]
    s2n = s2r[:8]
    c2n = c2r[:8]

    # conv1 block-diag lhsT per quad: [128, 32]
    w1bd = np.zeros((2, 128, 32), dtype=f32)
    for q in range(2):
        for gh in range(4):
            g = 4 * q + gh
            w1bd[q, 32 * gh : 32 * gh + 32, 8 * gh : 8 * gh + 8] = w1[g].T

    # fused conv2 weights with BN2 scales folded in
    w2m = np.zeros((64, 72), dtype=f32)
    for j in range(72):
        r = mperm[j]
        gc, co = r // 10, r % 10
        w2m[8 * gc : 8 * gc + 8, j] = w2[gc, co, :] * s2m[j]
    w2n = np.zeros((9, 8, 72), dtype=f32)
    for k in range(9):
        for g in range(8):
            gc, co = g // 10, g % 10      # ref ch g -> group 0, co g
            w2n[k, :, 8 * k + g] = w2[gc, co, :] * s2n[g]
    cf = (c2m + c2n[np.arange(72) % 8]).astype(f32)
    # boundary vector u: W_s^T u = -c2n, with W_s[ci, g] = w2[0, g, ci]*s2n[g]
    Ws = (w2[0, 0:8, :].T * s2n[None, :]).astype(np.float64)   # [ci, g]
    ubc = np.linalg.solve(Ws.T, -c2n.astype(np.float64)).astype(f32)

    # value conv block-diag lhsT per quad: [128, 128]
    wvbd = np.zeros((2, 128, 128), dtype=f32)
    for q in range(2):
        for gh in range(4):
            g = 4 * q + gh
            wvbd[q, 32 * gh : 32 * gh + 32, 32 * gh : 32 * gh + 32] = wv[g].T

    onesb = np.zeros((72, 8), dtype=ml_dtypes.bfloat16)
    for k in range(9):
        for g in range(8):
            onesb[8 * k + g, g] = 1
    rsel72 = np.zeros((8, 72), dtype=ml_dtypes.bfloat16)
    for k in range(9):
        for g in range(8):
            rsel72[g, 8 * k + g] = 1

    # packed const blocks
    cpkA = np.zeros((128, 320), dtype=ml_dtypes.bfloat16)
    cpkA[:, 0:32] = w1bd[0]
    cpkA[:, 32:64] = w1bd[1]
    cpkA[:, 64:192] = wvbd[0]
    cpkA[:, 192:320] = wvbd[1]
    cpkF = np.zeros((72, 721), dtype=f32)
    cpkF[0:64, 0:72] = w2m
    for k in range(9):
        cpkF[0:8, 72 + 72 * k : 144 + 72 * k] = w2n[k]
    cpkF[0:72, 720] = cf
    cpkB = np.zeros((72, 80), dtype=ml_dtypes.bfloat16)
    cpkB[0:72, 0:8] = onesb
    cpkB[0:8, 8:80] = rsel72

    # padded input: (2, 256, 130, 130)
    xp = np.zeros((B, C, H + 2, W + 2), dtype=f32)
    xp[:, :, 1:-1, 1:-1] = x

    shards = []
    for b in range(B):
        for qh in range(4):
            xs = xp[b, :, qh * HS : qh * HS + HI, :]       # [256, 34, 130]
            xs = np.ascontiguousarray(
                xs.reshape(2, 128, NIN).astype(ml_dtypes.bfloat16)
            )
            cpkS = np.zeros((64, 8), dtype=f32)
            cpkS[:, 0] = s1
            cpkS[:, 1] = c1
            if qh == 0:
                cpkS[0:8, 2] = 0.0
                cpkS[0:8, 3] = ubc
            else:
                cpkS[0:8, 2] = 1.0
                cpkS[0:8, 3] = 0.0
            if qh == 3:
                cpkS[0:8, 4] = 0.0
                cpkS[0:8, 5] = ubc
            else:
                cpkS[0:8, 4] = 1.0
                cpkS[0:8, 5] = 0.0
            cpkS[0:8, 6] = ubc
            shards.append(
                {
                    "x": xs,
                    "cpkA": cpkA, "cpkS": cpkS, "cpkF": cpkF, "cpkB": cpkB,
                }
            )
    return shards


def kernel(**inputs):
    if "nc" not in _NC_CACHE:
        _NC_CACHE["nc"] = _build_nc()
    nc = _NC_CACHE["nc"]

    shards = _host_prep(**inputs)
    res = run_bass_kernel_spmd(nc, shards, core_ids=list(range(8)))

    out = np.zeros((B, C, H, W), dtype=np.float32)
    for i, r in enumerate(res.results):
        b, qh = divmod(i, 4)
        o = r["out"].reshape(C, HS, WP)[:, :, 1 : 1 + W]
        out[b, :, qh * HS : (qh + 1) * HS, :] = o
    return out


# revision 36
# speedup vs baseline: 1.0537x; 1.0537x over previous
"""Trainium2 Bass kernel for nn_LocalAttn: grouped local attention (3x3 window).

Sharding: 8 cores = batch(2) x H-strips(4). Each core gets a 34-row slice
(32 output rows + 1 halo row each side) of the W-and-H zero-padded input,
so all cores run one identical SPMD program.

v2 design (channel-major, pixels on the free dim, W padded to 130):
  conv1 (PE, bf16 block-diag, x in bf16) -> BN1+bias+tanh (ScalarE) -> t
  (f32r). conv2 is FUSED into a direct 3x3 logits conv on the PE:
  logits[8k+g] = w2m'.t(center) + w2n'.t(+dk), PSUM-accumulated over the
  9 offsets with BN2 scales folded into the weights. Boundary cells of
  t's group-0 rows are set to u = solve(W2n_s^T u = -c2n) so the fused
  neighbor term exactly cancels the folded BN bias where the reference
  zero-pads. exp via ScalarE (bias = folded BN2 offset) -> e72 bf16;
  softmax denom via ones-matmul (PE) -> reciprocal (DVE) -> 1/denom
  broadcast over k via sel-matmul (PE) -> e72 normalized in place (DVE).
  Normalized e72 is staged to DRAM per fifth so broadcast DMAs (stride-0
  DRAM source dims) expand 8 group rows -> 128 channel rows straight
  into SBUF bf16 at pure DMA-queue cost. Apply, per (fifth, quad): 9
  broadcast DMAs on SP/ACT queues feed 9 multiplies (6 on DVE at bf16
  2x rate, 3 on Pool tensor_tensor), bf16 add tree on DVE, last add on
  Pool -> f32 out tile -> DMA out. Value conv (PE, bf16) -> ScalarE
  copies to bf16 v, interleaved with the conv phases.
"""

import numpy as np
import ml_dtypes

import concourse.bass as bass
import concourse.bacc as bacc
import concourse.mybir as mybir
from concourse import tile
from concourse.bass_utils import run_bass_kernel_spmd

F32 = mybir.dt.float32
F32R = mybir.dt.float32r
BF16 = mybir.dt.bfloat16
AF = mybir.ActivationFunctionType
ALU = mybir.AluOpType

EPS = 1e-5
G = 8          # groups
B = 2
C = 256
H = W = 128
HS = 32        # output rows per core
HI = 34        # input rows per core (with halo)
WP = 130       # padded width
NIN = HI * WP          # 4420
NOUT = HS * WP         # 4160
NPAD = NIN + 2         # t / v free size, data at base offset 1

# phase A/C row-chunking (PSUM free dim <= 512 f32)
ACH = 3                # rows per conv chunk
# apply stages (pixel offset, size): small first stage unlocks the apply
# as soon as the first conv2 chunk is done; small last stage shortens the
# drain. Rows per stage: 2, 6, 10, 10, 4.
STAGES = [(0, 260), (260, 780), (1040, 1300), (2340, 1300), (3640, 520)]
SMAX = 1300

# free-dim offset into a base-1 padded [.., NPAD] tensor for the (dy,dx)
# neighbor of output pixel 0 (= input row 1, col 0)
def _koff(k):
    dy, dx = k // 3 - 1, k % 3 - 1
    return 1 + WP + dy * WP + dx


# apply-phase mul ownership: which k's multiply on DVE vs Pool
KS_DVE = (0, 1, 2, 3, 4, 5)   # bf16 SBUF muls on DVE
KS_POOLS = (6, 7, 8)          # bf16 SBUF muls on Pool (tensor_tensor)

_NC_CACHE = {}


def _build_nc():
    nc = bacc.Bacc("TRN2", target_bir_lowering=False, debug=False, num_devices=8)

    x_d = nc.dram_tensor("x", [2, 128, NIN], BF16, kind="ExternalInput")
    ca_d = nc.dram_tensor("cpkA", [128, 320], BF16, kind="ExternalInput")
    cs_d = nc.dram_tensor("cpkS", [64, 8], F32, kind="ExternalInput")
    cf2_d = nc.dram_tensor("cpkF", [72, 721], F32R, kind="ExternalInput")
    cb_d = nc.dram_tensor("cpkB", [72, 80], BF16, kind="ExternalInput")
    out_d = nc.dram_tensor("out", [2, 128, NOUT], F32, kind="ExternalOutput")
    # DRAM scratch for broadcast staging
    e72_d = nc.dram_tensor("e72d", [72, NOUT], BF16, kind="Internal")

    nch = HI // ACH + (1 if HI % ACH else 0)       # 12 input chunks (11x3+1)
    ncho = HS // ACH + (1 if HS % ACH else 0)      # 11 output chunks (10x3+2)

    with tile.TileContext(nc) as tc:
        with (
            nc.allow_low_precision(reason="bf16 softmax weights/values"),
            tc.tile_pool(name="const", bufs=1) as cp,
            tc.tile_pool(name="mid", bufs=1) as mp,
        ):
            # ---- input loads first (SP/ACT), then weights ----
            xq = []
            for q in range(2):
                xt = mp.tile([128, NIN], BF16, tag=f"x_{q}", name=f"x_{q}")
                xq.append(xt)
            xrows = [(0, 4), (4, 9), (9, 16), (16, 25), (25, 34)]
            for ci, (ra, rb_) in enumerate(xrows):
                fsl = slice(ra * WP, rb_ * WP)
                for q in range(2):
                    eng = nc.sync if (ci + q) % 2 == 0 else nc.scalar
                    eng.dma_start(xq[q][:, fsl], x_d[q, :, fsl])

            # ---- constant / weight loads (packed, few DMAs) ----
            cA = cp.tile([128, 320], BF16, tag="cA", name="cA")
            nc.gpsimd.dma_start(cA[:], ca_d[:])
            w1t = [cA[:, 0:32], cA[:, 32:64]]
            wvt = [cA[:, 64:192], cA[:, 192:320]]
            cS = cp.tile([64, 8], F32, tag="cS", name="cS")
            nc.gpsimd.dma_start(cS[:], cs_d[:])
            s1t = cS[:, 0:1]
            c1t = cS[:, 1:2]
            atop = cS[0:8, 2:3]
            btop = cS[0:8, 3:4]
            abot = cS[0:8, 4:5]
            bbot = cS[0:8, 5:6]
            ubc = cS[0:8, 6:7]
            cF = cp.tile([72, 721], F32R, tag="cF", name="cF")
            nc.gpsimd.dma_start(cF[:], cf2_d[:])
            w2mt = cF[0:64, 0:72]
            w2nt = [cF[0:8, 72 + 72 * k : 144 + 72 * k] for k in range(9)]
            cft = cF[0:72, 720:721]
            cB = cp.tile([72, 80], BF16, tag="cB", name="cB")
            nc.gpsimd.dma_start(cB[:], cb_d[:])
            onest = cB[0:72, 0:8]
            r72t = cB[0:8, 8:80]

            # ---- persistent mid tensors ----
            t_sb = mp.tile([64, NPAD], F32R, tag="t", name="t")
            v_sb = [mp.tile([128, NPAD], BF16, tag=f"v_{q}", name=f"v_{q}") for q in range(2)]
            e72 = mp.tile([72, NOUT], BF16, tag="e72", name="e72")
            rb = mp.tile([8, NOUT], BF16, tag="rb", name="rb")

            # ---- phases A+C interleaved: conv1/tanh/vconv then fused
            # conv2/exp/denom two chunks behind, sharing one PSUM scope ----
            for q in range(2):
                nc.gpsimd.memset(v_sb[q][:, 0:1], 0.0)
                nc.gpsimd.memset(v_sb[q][:, NPAD - 1 : NPAD], 0.0)
            with (
                tc.tile_pool(name="pc64", bufs=2, space="PSUM") as pc64,
                tc.tile_pool(name="pv", bufs=2, space="PSUM") as pvp,
                tc.tile_pool(name="pcm", bufs=2, space="PSUM") as pcm,
                tc.tile_pool(name="pcd", bufs=1, space="PSUM") as pcd,
                tc.tile_pool(name="pcr", bufs=1, space="PSUM") as pcr,
            ):
                def a_chunk(ch):
                    r0 = ch * ACH
                    r1 = min(r0 + ACH, HI)
                    f0 = r0 * WP
                    fsz = (r1 - r0) * WP
                    sl = slice(f0, f0 + fsz)
                    pt = pc64.tile([64, fsz], F32, tag="pc", name="pc", padded_shape=[64, 512])
                    nc.tensor.matmul(
                        pt[0:32, :], w1t[0],
                        xq[0][:, sl],
                        start=True, stop=True, tile_position=(0, 0),
                    )
                    nc.tensor.matmul(
                        pt[32:64, :], w1t[1],
                        xq[1][:, sl],
                        start=True, stop=True, tile_position=(0, 32),
                    )
                    nc.scalar.activation(
                        t_sb[:, 1 + f0 : 1 + f0 + fsz], pt[:],
                        AF.Tanh, bias=c1t, scale=s1t,
                    )
                    # boundary cells of t (group-0 rows): halo rows become
                    # t*a + b (a,b host-set: u at image boundary, identity
                    # elsewhere); W-pad columns always become u, so the fused
                    # neighbor term cancels the folded BN bias off-image
                    if ch == 0:
                        nc.vector.tensor_scalar(
                            t_sb[0:8, 1 : 1 + WP], t_sb[0:8, 1 : 1 + WP],
                            atop, btop, ALU.mult, ALU.add,
                        )
                    if ch == nch - 1:
                        nc.vector.tensor_scalar(
                            t_sb[0:8, 1 + 33 * WP : 1 + 34 * WP],
                            t_sb[0:8, 1 + 33 * WP : 1 + 34 * WP],
                            abot, bbot, ALU.mult, ALU.add,
                        )
                    nc.vector.tensor_scalar(
                        t_sb[0:8, f0 : f0 + fsz : WP],
                        xq[0][0:8, 0 : fsz : WP],
                        0.0, ubc, ALU.mult, ALU.add,
                    )
                    nc.vector.tensor_scalar(
                        t_sb[0:8, 1 + f0 : 1 + f0 + fsz : WP],
                        xq[0][0:8, 0 : fsz : WP],
                        0.0, ubc, ALU.mult, ALU.add,
                    )
                    if ch == nch - 1:
                        nc.vector.tensor_scalar(
                            t_sb[0:8, f0 + fsz : NPAD],
                            xq[0][0:8, 0 : NPAD - f0 - fsz],
                            0.0, ubc, ALU.mult, ALU.add,
                        )
                def v_chunk(ch):
                    r0 = ch * ACH
                    r1 = min(r0 + ACH, HI)
                    f0 = r0 * WP
                    fsz = (r1 - r0) * WP
                    sl = slice(f0, f0 + fsz)
                    for q in range(2):
                        pv = pvp.tile([128, fsz], F32, tag="pv", name="pv", padded_shape=[128, 512])
                        nc.tensor.matmul(
                            pv[:], wvt[q],
                            xq[q][:, sl],
                            start=True, stop=True,
                        )
                        nc.scalar.copy(v_sb[q][:, 1 + f0 : 1 + f0 + fsz], pv[:])

                def c_chunk(ch):
                    r0 = ch * ACH
                    r1 = min(r0 + ACH, HS)
                    fsz = (r1 - r0) * WP
                    o0 = r0 * WP                     # offset in out space
                    tbase = 1 + WP + o0              # center in t space
                    pm = pcm.tile([72, fsz], F32, tag="pm", name="pm", padded_shape=[72, 512])
                    nc.tensor.matmul(
                        pm[:], w2mt,
                        t_sb[:, tbase : tbase + fsz],
                        start=True, stop=False, skip_group_check=True,
                    )
                    for k in range(9):
                        dy, dx = k // 3 - 1, k % 3 - 1
                        tb = tbase + dy * WP + dx
                        nc.tensor.matmul(
                            pm[:],
                            w2nt[k],
                            t_sb[0:8, tb : tb + fsz],
                            start=False, stop=(k == 8), skip_group_check=True,
                        )
                    nc.scalar.activation(
                        e72[:, o0 : o0 + fsz], pm[:],
                        AF.Exp, bias=cft,
                    )
                    pd = pcd.tile([8, fsz], F32, tag="pd", name="pd", padded_shape=[8, 512])
                    nc.tensor.matmul(
                        pd[:], onest, e72[:, o0 : o0 + fsz],
                        start=True, stop=True,
                    )
                    nc.vector.reciprocal(rb[:, o0 : o0 + fsz], pd[:])
                    pr72 = pcr.tile([72, fsz], F32, tag="pr72", name="pr72", padded_shape=[72, 512])
                    nc.tensor.matmul(
                        pr72[:], r72t, rb[:, o0 : o0 + fsz],
                        start=True, stop=True,
                    )
                    nc.vector.tensor_mul(
                        e72[:, o0 : o0 + fsz], e72[:, o0 : o0 + fsz], pr72[:]
                    )

                LAG = 2
                for ch in range(nch + ncho):
                    if ch < nch:
                        a_chunk(ch)
                    if LAG <= ch and ch - LAG < ncho:
                        c_chunk(ch - LAG)
                    if LAG <= ch and ch - LAG < nch:
                        v_chunk(ch - LAG)
                for ch in range(nch - LAG, nch):
                    v_chunk(ch)

            # stage normalized e72 to DRAM (for the pab broadcasts)
            for h0, sz in STAGES:
                nc.sync.dma_start(e72_d[:, h0 : h0 + sz], e72[:, h0 : h0 + sz])

            # ---- phase D: apply ----
            with (
                nc.allow_low_precision(reason="3x3 softmax-weighted sum in bf16"),
                tc.tile_pool(name="pab", bufs=14) as pabp,
                tc.tile_pool(name="prod", bufs=11) as prp,
                tc.tile_pool(name="sums", bufs=6) as smp,
                tc.tile_pool(name="outp", bufs=3) as outp,
            ):
                for h, (h0, sz) in enumerate(STAGES):
                    for q in range(2):
                        hsl = slice(h0, h0 + sz)
                        pab = {}
                        for k in range(9):
                            pt = pabp.tile([128, SMAX], BF16, tag="pab", name="pab")
                            src_ap = (
                                e72_d[8 * k + 4 * q : 8 * k + 4 * q + 4, hsl]
                                .unsqueeze(1).broadcast_to([4, 32, sz])
                            )
                            eng = nc.sync if k in (0, 2, 4, 6, 7, 8) else nc.scalar
                            eng.dma_start(pt[:, 0:sz], src_ap)
                            pab[k] = pt
                        prod = {}
                        for k in range(9):
                            voff = _koff(k) + h0
                            vsl = v_sb[q][:, voff : voff + sz]
                            pr = prp.tile([128, SMAX], BF16, tag="pr", name="pr")
                            if k in KS_DVE:
                                nc.vector.tensor_mul(pr[:, 0:sz], pab[k][:, 0:sz], vsl)
                            else:
                                nc.gpsimd.tensor_mul(pr[:, 0:sz], pab[k][:, 0:sz], vsl)
                            prod[k] = pr
                        # balanced bf16 add tree on DVE; final add on Pool
                        s01 = smp.tile([128, SMAX], BF16, tag="s", name="s01")
                        nc.vector.tensor_add(s01[:, 0:sz], prod[0][:, 0:sz], prod[1][:, 0:sz])
                        s23 = smp.tile([128, SMAX], BF16, tag="s", name="s23")
                        nc.vector.tensor_add(s23[:, 0:sz], prod[2][:, 0:sz], prod[3][:, 0:sz])
                        s45 = smp.tile([128, SMAX], BF16, tag="s", name="s45")
                        nc.vector.tensor_add(s45[:, 0:sz], prod[4][:, 0:sz], prod[5][:, 0:sz])
                        nc.vector.tensor_add(s01[:, 0:sz], s01[:, 0:sz], s23[:, 0:sz])
                        nc.vector.tensor_add(s01[:, 0:sz], s01[:, 0:sz], s45[:, 0:sz])
                        nc.vector.tensor_add(s01[:, 0:sz], s01[:, 0:sz], prod[6][:, 0:sz])
                        nc.vector.tensor_add(s01[:, 0:sz], s01[:, 0:sz], prod[7][:, 0:sz])
                        ot = outp.tile([128, SMAX], F32, tag="ot", name="ot")
                        nc.gpsimd.tensor_add(ot[:, 0:sz], s01[:, 0:sz], prod[8][:, 0:sz])
                        nc.gpsimd.dma_start(out_d[q, :, hsl], ot[:, 0:sz])

    nc.compile()
    return nc


def _host_prep(x, w1, b1, g1, be1, m1, v1, w2, b2, g2, be2, m2, v2, wv):
    f32 = np.float32

    inv1 = (g1 / np.sqrt(v1 + EPS)).astype(f32)            # [64]
    s1 = inv1
    c1 = (b1 * inv1 + be1 - m1 * inv1).astype(f32)
    inv2 = (g2 / np.sqrt(v2 + EPS)).astype(f32)            # [80]
    s2r = inv2
    c2r = (b2 * inv2 + be2 - m2 * inv2).astype(f32)

    # conv2 output layout: psum row j = 8k+g -> ref mask ch 8+9g+k;
    # neighbor path: ref ch g (g<8), i.e. group 0 of t, co=g
    mperm = np.zeros(72, dtype=np.int64)
    for k in range(9):
        for g in range(8):
            mperm[8 * k + g] = 8 + 9 * g + k
    s2m = s2r[mperm]
    c2m = c2r[mperm]
    s2n = s2r[:8]
    c2n = c2r[:8]

    # conv1 block-diag lhsT per quad: [128, 32]
    w1bd = np.zeros((2, 128, 32), dtype=f32)
    for q in range(2):
        for gh in range(4):
            g = 4 * q + gh
            w1bd[q, 32 * gh : 32 * gh + 32, 8 * gh : 8 * gh + 8] = w1[g].T

    # fused conv2 weights with BN2 scales folded in
    w2m = np.zeros((64, 72), dtype=f32)
    for j in range(72):
        r = mperm[j]
        gc, co = r // 10, r % 10
        w2m[8 * gc : 8 * gc + 8, j] = w2[gc, co, :] * s2m[j]
    w2n = np.zeros((9, 8, 72), dtype=f32)
    for k in range(9):
        for g in range(8):
            gc, co = g // 10, g % 10      # ref ch g -> group 0, co g
            w2n[k, :, 8 * k + g] = w2[gc, co, :] * s2n[g]
    cf = (c2m + c2n[np.arange(72) % 8]).astype(f32)
    # boundary vector u: W_s^T u = -c2n, with W_s[ci, g] = w2[0, g, ci]*s2n[g]
    Ws = (w2[0, 0:8, :].T * s2n[None, :]).astype(np.float64)   # [ci, g]
    ubc = np.linalg.solve(Ws.T, -c2n.astype(np.float64)).astype(f32)

    # value conv block-diag lhsT per quad: [128, 128]
    wvbd = np.zeros((2, 128, 128), dtype=f32)
    for q in range(2):
        for gh in range(4):
            g = 4 * q + gh
            wvbd[q, 32 * gh : 32 * gh + 32, 32 * gh : 32 * gh + 32] = wv[g].T

    onesb = np.zeros((72, 8), dtype=ml_dtypes.bfloat16)
    for k in range(9):
        for g in range(8):
            onesb[8 * k + g, g] = 1
    rsel72 = np.zeros((8, 72), dtype=ml_dtypes.bfloat16)
    for k in range(9):
        for g in range(8):
            rsel72[g, 8 * k + g] = 1

    # packed const blocks
    cpkA = np.zeros((128, 320), dtype=ml_dtypes.bfloat16)
    cpkA[:, 0:32] = w1bd[0]
    cpkA[:, 32:64] = w1bd[1]
    cpkA[:, 64:192] = wvbd[0]
    cpkA[:, 192:320] = wvbd[1]
    cpkF = np.zeros((72, 721), dtype=f32)
    cpkF[0:64, 0:72] = w2m
    for k in range(9):
        cpkF[0:8, 72 + 72 * k : 144 + 72 * k] = w2n[k]
    cpkF[0:72, 720] = cf
    cpkB = np.zeros((72, 80), dtype=ml_dtypes.bfloat16)
    cpkB[0:72, 0:8] = onesb
    cpkB[0:8, 8:80] = rsel72

    # padded input: (2, 256, 130, 130)
    xp = np.zeros((B, C, H + 2, W + 2), dtype=f32)
    xp[:, :, 1:-1, 1:-1] = x

    shards = []
    for b in range(B):
        for qh in range(4):
            xs = xp[b, :, qh * HS : qh * HS + HI, :]       # [256, 34, 130]
            xs = np.ascontiguousarray(
                xs.reshape(2, 128, NIN).astype(ml_dtypes.bfloat16)
            )
            cpkS = np.zeros((64, 8), dtype=f32)
            cpkS[:, 0] = s1
            cpkS[:, 1] = c1
            if qh == 0:
                cpkS[0:8, 2] = 0.0
                cpkS[0:8, 3] = ubc
            else:
                cpkS[0:8, 2] = 1.0
                cpkS[0:8, 3] = 0.0
            if qh == 3:
                cpkS[0:8, 4] = 0.0
                cpkS[0:8, 5] = ubc
            else:
                cpkS[0:8, 4] = 1.0
                cpkS[0:8, 5] = 0.0
            cpkS[0:8, 6] = ubc
            shards.append(
                {
                    "x": xs,
                    "cpkA": cpkA, "cpkS": cpkS, "cpkF": cpkF, "cpkB": cpkB,
                }
            )
    return shards


def kernel(**inputs):
    if "nc" not in _NC_CACHE:
        _NC_CACHE["nc"] = _build_nc()
    nc = _NC_CACHE["nc"]

    shards = _host_prep(**inputs)
    res = run_bass_kernel_spmd(nc, shards, core_ids=list(range(8)))

    out = np.zeros((B, C, H, W), dtype=np.float32)
    for i, r in enumerate(res.results):
        b, qh = divmod(i, 4)
        o = r["out"].reshape(C, HS, WP)[:, :, 1 : 1 + W]
        out[b, :, qh * HS : (qh + 1) * HS, :] = o
    return out


# revision 44
# speedup vs baseline: 1.0947x; 1.0389x over previous
"""Trainium2 Bass kernel for nn_LocalAttn: grouped local attention (3x3 window).

Sharding: 8 cores = batch(2) x H-strips(4). Each core gets a 34-row slice
(32 output rows + 1 halo row each side) of the W-and-H zero-padded input,
so all cores run one identical SPMD program.

v2 design (channel-major, pixels on the free dim, W padded to 130):
  conv1 (PE, bf16 block-diag, x in bf16) -> BN1+bias+tanh (ScalarE) -> t
  (f32r). conv2 is FUSED into a direct 3x3 logits conv on the PE:
  logits[8k+g] = w2m'.t(center) + w2n'.t(+dk), PSUM-accumulated over the
  9 offsets with BN2 scales folded into the weights. Boundary cells of
  t's group-0 rows are set to u = solve(W2n_s^T u = -c2n) so the fused
  neighbor term exactly cancels the folded BN bias where the reference
  zero-pads. exp via ScalarE (bias = folded BN2 offset) -> e72 bf16;
  softmax denom via ones-matmul (PE) -> reciprocal (DVE) -> 1/denom
  broadcast over k via sel-matmul (PE) -> e72 normalized in place (DVE).
  Normalized e72 is staged to DRAM per fifth so broadcast DMAs (stride-0
  DRAM source dims) expand 8 group rows -> 128 channel rows straight
  into SBUF bf16 at pure DMA-queue cost. Apply, per (fifth, quad): 9
  broadcast DMAs on SP/ACT queues feed 9 multiplies (6 on DVE at bf16
  2x rate, 3 on Pool tensor_tensor), bf16 add tree on DVE, last add on
  Pool -> f32 out tile -> DMA out. Value conv (PE, bf16) -> ScalarE
  copies to bf16 v, interleaved with the conv phases.
"""

import numpy as np
import ml_dtypes

import concourse.bass as bass
import concourse.bacc as bacc
import concourse.mybir as mybir
from concourse import tile
from concourse.bass_utils import run_bass_kernel_spmd

F32 = mybir.dt.float32
F32R = mybir.dt.float32r
BF16 = mybir.dt.bfloat16
AF = mybir.ActivationFunctionType
ALU = mybir.AluOpType

EPS = 1e-5
G = 8          # groups
B = 2
C = 256
H = W = 128
HS = 32        # output rows per core
HI = 34        # input rows per core (with halo)
WP = 130       # padded width
NIN = HI * WP          # 4420
NOUT = HS * WP         # 4160
NPAD = NIN + 2         # t / v free size, data at base offset 1

# phase A/C row-chunking (PSUM free dim <= 512 f32)
ACH = 3                # rows per conv chunk
# apply stages (pixel offset, size): small first stage unlocks the apply
# as soon as the first conv2 chunk is done; small last stage shortens the
# drain. Rows per stage: 2, 6, 10, 10, 4.
STAGES = [(0, 260), (260, 780), (1040, 1300), (2340, 1300), (3640, 520)]
SMAX = 1300

# free-dim offset into a base-1 padded [.., NPAD] tensor for the (dy,dx)
# neighbor of output pixel 0 (= input row 1, col 0)
def _koff(k):
    dy, dx = k // 3 - 1, k % 3 - 1
    return 1 + WP + dy * WP + dx


# apply-phase mul ownership: which k's multiply on DVE vs Pool
KS_DVE = (0, 1, 2, 3, 4, 5)   # bf16 SBUF muls on DVE
KS_POOLS = (6, 7, 8)          # bf16 SBUF muls on Pool (tensor_tensor)

_NC_CACHE = {}


def _build_nc():
    nc = bacc.Bacc("TRN2", target_bir_lowering=False, debug=False, num_devices=8)

    x_d = nc.dram_tensor("x", [2, 128, NIN], BF16, kind="ExternalInput")
    ca_d = nc.dram_tensor("cpkA", [128, 320], BF16, kind="ExternalInput")
    cs_d = nc.dram_tensor("cpkS", [64, 8], F32, kind="ExternalInput")
    cf2_d = nc.dram_tensor("cpkF", [72, 721], F32R, kind="ExternalInput")
    cb_d = nc.dram_tensor("cpkB", [72, 80], BF16, kind="ExternalInput")
    out_d = nc.dram_tensor("out", [2, 128, NOUT], F32, kind="ExternalOutput")
    # DRAM scratch for broadcast staging
    e72_d = nc.dram_tensor("e72d", [72, NOUT], BF16, kind="Internal")

    nch = HI // ACH + (1 if HI % ACH else 0)       # 12 input chunks (11x3+1)
    ncho = HS // ACH + (1 if HS % ACH else 0)      # 11 output chunks (10x3+2)

    with tile.TileContext(nc) as tc:
        with (
            nc.allow_low_precision(reason="bf16 softmax weights/values"),
            tc.tile_pool(name="const", bufs=1) as cp,
            tc.tile_pool(name="mid", bufs=1) as mp,
        ):
            # ---- input loads first (SP/ACT), then weights ----
            xq = []
            for q in range(2):
                xt = mp.tile([128, NIN], BF16, tag=f"x_{q}", name=f"x_{q}")
                xq.append(xt)
            xrows = [(0, 4), (4, 9), (9, 16), (16, 25), (25, 34)]
            for ci, (ra, rb_) in enumerate(xrows):
                fsl = slice(ra * WP, rb_ * WP)
                for q in range(2):
                    eng = nc.sync if (ci + q) % 2 == 0 else nc.scalar
                    eng.dma_start(xq[q][:, fsl], x_d[q, :, fsl])

            # ---- constant / weight loads (packed, few DMAs) ----
            cA = cp.tile([128, 320], BF16, tag="cA", name="cA")
            nc.gpsimd.dma_start(cA[:], ca_d[:])
            w1t = [cA[:, 0:32], cA[:, 32:64]]
            wvt = [cA[:, 64:192], cA[:, 192:320]]
            cS = cp.tile([64, 8], F32, tag="cS", name="cS")
            nc.gpsimd.dma_start(cS[:], cs_d[:])
            s1t = cS[:, 0:1]
            c1t = cS[:, 1:2]
            atop = cS[0:8, 2:3]
            btop = cS[0:8, 3:4]
            abot = cS[0:8, 4:5]
            bbot = cS[0:8, 5:6]
            ubc = cS[0:8, 6:7]
            cF = cp.tile([72, 721], F32R, tag="cF", name="cF")
            nc.gpsimd.dma_start(cF[:], cf2_d[:])
            w2mt = cF[0:64, 0:72]
            w2nt = [cF[0:8, 72 + 72 * k : 144 + 72 * k] for k in range(9)]
            cft = cF[0:72, 720:721]
            cB = cp.tile([72, 80], BF16, tag="cB", name="cB")
            nc.gpsimd.dma_start(cB[:], cb_d[:])
            onest = cB[0:72, 0:8]
            r72t = cB[0:8, 8:80]

            # ---- persistent mid tensors ----
            t_sb = mp.tile([64, NPAD], F32R, tag="t", name="t")
            v_sb = [mp.tile([128, NPAD], BF16, tag=f"v_{q}", name=f"v_{q}") for q in range(2)]
            e72 = mp.tile([72, NOUT], BF16, tag="e72", name="e72")
            rb = mp.tile([8, NOUT], BF16, tag="rb", name="rb")

            # ---- phases A+C interleaved: conv1/tanh/vconv then fused
            # conv2/exp/denom two chunks behind, sharing one PSUM scope ----
            for q in range(2):
                nc.gpsimd.memset(v_sb[q][:, 0:1], 0.0)
                nc.gpsimd.memset(v_sb[q][:, NPAD - 1 : NPAD], 0.0)
            with (
                tc.tile_pool(name="pc64", bufs=2, space="PSUM") as pc64,
                tc.tile_pool(name="pv", bufs=2, space="PSUM") as pvp,
                tc.tile_pool(name="pcm", bufs=2, space="PSUM") as pcm,
                tc.tile_pool(name="pcd", bufs=1, space="PSUM") as pcd,
                tc.tile_pool(name="pcr", bufs=1, space="PSUM") as pcr,
            ):
                def a_chunk(ch):
                    r0 = ch * ACH
                    r1 = min(r0 + ACH, HI)
                    f0 = r0 * WP
                    fsz = (r1 - r0) * WP
                    sl = slice(f0, f0 + fsz)
                    pt = pc64.tile([64, fsz], F32, tag="pc", name="pc", padded_shape=[64, 512])
                    nc.tensor.matmul(
                        pt[0:32, :], w1t[0],
                        xq[0][:, sl],
                        start=True, stop=True, tile_position=(0, 0),
                    )
                    nc.tensor.matmul(
                        pt[32:64, :], w1t[1],
                        xq[1][:, sl],
                        start=True, stop=True, tile_position=(0, 32),
                    )
                    nc.scalar.activation(
                        t_sb[:, 1 + f0 : 1 + f0 + fsz], pt[:],
                        AF.Tanh, bias=c1t, scale=s1t,
                    )
                    # boundary cells of t (group-0 rows): halo rows become
                    # t*a + b (a,b host-set: u at image boundary, identity
                    # elsewhere); W-pad columns always become u, so the fused
                    # neighbor term cancels the folded BN bias off-image
                    if ch == 0:
                        nc.vector.tensor_scalar(
                            t_sb[0:8, 1 : 1 + WP], t_sb[0:8, 1 : 1 + WP],
                            atop, btop, ALU.mult, ALU.add,
                        )
                    if ch == nch - 1:
                        nc.vector.tensor_scalar(
                            t_sb[0:8, 1 + 33 * WP : 1 + 34 * WP],
                            t_sb[0:8, 1 + 33 * WP : 1 + 34 * WP],
                            abot, bbot, ALU.mult, ALU.add,
                        )
                    nc.vector.tensor_scalar(
                        t_sb[0:8, f0 : f0 + fsz : WP],
                        xq[0][0:8, 0 : fsz : WP],
                        0.0, ubc, ALU.mult, ALU.add,
                    )
                    nc.vector.tensor_scalar(
                        t_sb[0:8, 1 + f0 : 1 + f0 + fsz : WP],
                        xq[0][0:8, 0 : fsz : WP],
                        0.0, ubc, ALU.mult, ALU.add,
                    )
                    if ch == nch - 1:
                        nc.vector.tensor_scalar(
                            t_sb[0:8, f0 + fsz : NPAD],
                            xq[0][0:8, 0 : NPAD - f0 - fsz],
                            0.0, ubc, ALU.mult, ALU.add,
                        )
                def v_chunk(ch):
                    r0 = ch * ACH
                    r1 = min(r0 + ACH, HI)
                    f0 = r0 * WP
                    fsz = (r1 - r0) * WP
                    sl = slice(f0, f0 + fsz)
                    for q in range(2):
                        pv = pvp.tile([128, fsz], F32, tag="pv", name="pv", padded_shape=[128, 512])
                        nc.tensor.matmul(
                            pv[:], wvt[q],
                            xq[q][:, sl],
                            start=True, stop=True,
                        )
                        # early chunks: copy on DVE (idle during the ramp),
                        # late chunks: ACT (DVE saturated by then)
                        if ch < 3:
                            nc.vector.tensor_copy(
                                v_sb[q][:, 1 + f0 : 1 + f0 + fsz], pv[:]
                            )
                        else:
                            nc.scalar.copy(v_sb[q][:, 1 + f0 : 1 + f0 + fsz], pv[:])

                def c_chunk(ch):
                    r0 = ch * ACH
                    r1 = min(r0 + ACH, HS)
                    fsz = (r1 - r0) * WP
                    o0 = r0 * WP                     # offset in out space
                    tbase = 1 + WP + o0              # center in t space
                    pm = pcm.tile([72, fsz], F32, tag="pm", name="pm", padded_shape=[72, 512])
                    nc.tensor.matmul(
                        pm[:], w2mt,
                        t_sb[:, tbase : tbase + fsz],
                        start=True, stop=False, skip_group_check=True,
                    )
                    for k in range(9):
                        dy, dx = k // 3 - 1, k % 3 - 1
                        tb = tbase + dy * WP + dx
                        nc.tensor.matmul(
                            pm[:],
                            w2nt[k],
                            t_sb[0:8, tb : tb + fsz],
                            start=False, stop=(k == 8), skip_group_check=True,
                        )
                    nc.scalar.activation(
                        e72[:, o0 : o0 + fsz], pm[:],
                        AF.Exp, bias=cft,
                    )
                    pd = pcd.tile([8, fsz], F32, tag="pd", name="pd", padded_shape=[8, 512])
                    nc.tensor.matmul(
                        pd[:], onest, e72[:, o0 : o0 + fsz],
                        start=True, stop=True,
                    )
                    nc.vector.reciprocal(rb[:, o0 : o0 + fsz], pd[:])
                    pr72 = pcr.tile([72, fsz], F32, tag="pr72", name="pr72", padded_shape=[72, 512])
                    nc.tensor.matmul(
                        pr72[:], r72t, rb[:, o0 : o0 + fsz],
                        start=True, stop=True,
                    )
                    nc.vector.tensor_mul(
                        e72[:, o0 : o0 + fsz], e72[:, o0 : o0 + fsz], pr72[:]
                    )

                LAG = 2
                VLAG = 4
                for ch in range(nch + ncho):
                    if ch < nch:
                        a_chunk(ch)
                    if LAG <= ch and ch - LAG < ncho:
                        c_chunk(ch - LAG)
                    if VLAG <= ch and ch - VLAG < nch:
                        v_chunk(ch - VLAG)
                for ch in range(nch - VLAG, nch):
                    v_chunk(ch)

            # stage normalized e72 to DRAM (for the pab broadcasts)
            for h0, sz in STAGES:
                nc.sync.dma_start(e72_d[:, h0 : h0 + sz], e72[:, h0 : h0 + sz])

            # ---- phase D: apply ----
            with (
                nc.allow_low_precision(reason="3x3 softmax-weighted sum in bf16"),
                tc.tile_pool(name="pab", bufs=14) as pabp,
                tc.tile_pool(name="prod", bufs=11) as prp,
                tc.tile_pool(name="sums", bufs=6) as smp,
                tc.tile_pool(name="outp", bufs=3) as outp,
            ):
                for h, (h0, sz) in enumerate(STAGES):
                    for q in range(2):
                        hsl = slice(h0, h0 + sz)
                        pab = {}
                        for k in range(9):
                            pt = pabp.tile([128, SMAX], BF16, tag="pab", name="pab")
                            src_ap = (
                                e72_d[8 * k + 4 * q : 8 * k + 4 * q + 4, hsl]
                                .unsqueeze(1).broadcast_to([4, 32, sz])
                            )
                            eng = nc.sync if k in (0, 2, 4, 6, 7, 8) else nc.scalar
                            eng.dma_start(pt[:, 0:sz], src_ap)
                            pab[k] = pt
                        prod = {}
                        for k in range(9):
                            voff = _koff(k) + h0
                            vsl = v_sb[q][:, voff : voff + sz]
                            pr = prp.tile([128, SMAX], BF16, tag="pr", name="pr")
                            if k in KS_DVE:
                                nc.vector.tensor_mul(pr[:, 0:sz], pab[k][:, 0:sz], vsl)
                            else:
                                nc.gpsimd.tensor_mul(pr[:, 0:sz], pab[k][:, 0:sz], vsl)
                            prod[k] = pr
                        # balanced bf16 add tree on DVE; final add on Pool
                        s01 = smp.tile([128, SMAX], BF16, tag="s", name="s01")
                        nc.vector.tensor_add(s01[:, 0:sz], prod[0][:, 0:sz], prod[1][:, 0:sz])
                        s23 = smp.tile([128, SMAX], BF16, tag="s", name="s23")
                        nc.vector.tensor_add(s23[:, 0:sz], prod[2][:, 0:sz], prod[3][:, 0:sz])
                        s45 = smp.tile([128, SMAX], BF16, tag="s", name="s45")
                        nc.vector.tensor_add(s45[:, 0:sz], prod[4][:, 0:sz], prod[5][:, 0:sz])
                        nc.vector.tensor_add(s01[:, 0:sz], s01[:, 0:sz], s23[:, 0:sz])
                        nc.vector.tensor_add(s01[:, 0:sz], s01[:, 0:sz], s45[:, 0:sz])
                        nc.vector.tensor_add(s01[:, 0:sz], s01[:, 0:sz], prod[6][:, 0:sz])
                        nc.vector.tensor_add(s01[:, 0:sz], s01[:, 0:sz], prod[7][:, 0:sz])
                        ot = outp.tile([128, SMAX], F32, tag="ot", name="ot")
                        nc.gpsimd.tensor_add(ot[:, 0:sz], s01[:, 0:sz], prod[8][:, 0:sz])
                        nc.gpsimd.dma_start(out_d[q, :, hsl], ot[:, 0:sz])

    nc.compile()
    return nc


def _host_prep(x, w1, b1, g1, be1, m1, v1, w2, b2, g2, be2, m2, v2, wv):
    f32 = np.float32

    inv1 = (g1 / np.sqrt(v1 + EPS)).astype(f32)            # [64]
    s1 = inv1
    c1 = (b1 * inv1 + be1 - m1 * inv1).astype(f32)
    inv2 = (g2 / np.sqrt(v2 + EPS)).astype(f32)            # [80]
    s2r = inv2
    c2r = (b2 * inv2 + be2 - m2 * inv2).astype(f32)

    # conv2 output layout: psum row j = 8k+g -> ref mask ch 8+9g+k;
    # neighbor path: ref ch g (g<8), i.e. group 0 of t, co=g
    mperm = np.zeros(72, dtype=np.int64)
    for k in range(9):
        for g in range(8):
            mperm[8 * k + g] = 8 + 9 * g + k
    s2m = s2r[mperm]
    c2m = c2r[mperm]
    s2n = s2r[:8]
    c2n = c2r[:8]

    # conv1 block-diag lhsT per quad: [128, 32]
    w1bd = np.zeros((2, 128, 32), dtype=f32)
    for q in range(2):
        for gh in range(4):
            g = 4 * q + gh
            w1bd[q, 32 * gh : 32 * gh + 32, 8 * gh : 8 * gh + 8] = w1[g].T

    # fused conv2 weights with BN2 scales folded in
    w2m = np.zeros((64, 72), dtype=f32)
    for j in range(72):
        r = mperm[j]
        gc, co = r // 10, r % 10
        w2m[8 * gc : 8 * gc + 8, j] = w2[gc, co, :] * s2m[j]
    w2n = np.zeros((9, 8, 72), dtype=f32)
    for k in range(9):
        for g in range(8):
            gc, co = g // 10, g % 10      # ref ch g -> group 0, co g
            w2n[k, :, 8 * k + g] = w2[gc, co, :] * s2n[g]
    cf = (c2m + c2n[np.arange(72) % 8]).astype(f32)
    # boundary vector u: W_s^T u = -c2n, with W_s[ci, g] = w2[0, g, ci]*s2n[g]
    Ws = (w2[0, 0:8, :].T * s2n[None, :]).astype(np.float64)   # [ci, g]
    ubc = np.linalg.solve(Ws.T, -c2n.astype(np.float64)).astype(f32)

    # value conv block-diag lhsT per quad: [128, 128]
    wvbd = np.zeros((2, 128, 128), dtype=f32)
    for q in range(2):
        for gh in range(4):
            g = 4 * q + gh
            wvbd[q, 32 * gh : 32 * gh + 32, 32 * gh : 32 * gh + 32] = wv[g].T

    onesb = np.zeros((72, 8), dtype=ml_dtypes.bfloat16)
    for k in range(9):
        for g in range(8):
            onesb[8 * k + g, g] = 1
    rsel72 = np.zeros((8, 72), dtype=ml_dtypes.bfloat16)
    for k in range(9):
        for g in range(8):
            rsel72[g, 8 * k + g] = 1

    # packed const blocks
    cpkA = np.zeros((128, 320), dtype=ml_dtypes.bfloat16)
    cpkA[:, 0:32] = w1bd[0]
    cpkA[:, 32:64] = w1bd[1]
    cpkA[:, 64:192] = wvbd[0]
    cpkA[:, 192:320] = wvbd[1]
    cpkF = np.zeros((72, 721), dtype=f32)
    cpkF[0:64, 0:72] = w2m
    for k in range(9):
        cpkF[0:8, 72 + 72 * k : 144 + 72 * k] = w2n[k]
    cpkF[0:72, 720] = cf
    cpkB = np.zeros((72, 80), dtype=ml_dtypes.bfloat16)
    cpkB[0:72, 0:8] = onesb
    cpkB[0:8, 8:80] = rsel72

    # padded input: (2, 256, 130, 130)
    xp = np.zeros((B, C, H + 2, W + 2), dtype=f32)
    xp[:, :, 1:-1, 1:-1] = x

    shards = []
    for b in range(B):
        for qh in range(4):
            xs = xp[b, :, qh * HS : qh * HS + HI, :]       # [256, 34, 130]
            xs = np.ascontiguousarray(
                xs.reshape(2, 128, NIN).astype(ml_dtypes.bfloat16)
            )
            cpkS = np.zeros((64, 8), dtype=f32)
            cpkS[:, 0] = s1
            cpkS[:, 1] = c1
            if qh == 0:
                cpkS[0:8, 2] = 0.0
                cpkS[0:8, 3] = ubc
            else:
                cpkS[0:8, 2] = 1.0
                cpkS[0:8, 3] = 0.0
            if qh == 3:
                cpkS[0:8, 4] = 0.0
                cpkS[0:8, 5] = ubc
            else:
                cpkS[0:8, 4] = 1.0
                cpkS[0:8, 5] = 0.0
            cpkS[0:8, 6] = ubc
            shards.append(
                {
                    "x": xs,
                    "cpkA": cpkA, "cpkS": cpkS, "cpkF": cpkF, "cpkB": cpkB,
                }
            )
    return shards


def kernel(**inputs):
    if "nc" not in _NC_CACHE:
        _NC_CACHE["nc"] = _build_nc()
    nc = _NC_CACHE["nc"]

    shards = _host_prep(**inputs)
    res = run_bass_kernel_spmd(nc, shards, core_ids=list(range(8)))

    out = np.zeros((B, C, H, W), dtype=np.float32)
    for i, r in enumerate(res.results):
        b, qh = divmod(i, 4)
        o = r["out"].reshape(C, HS, WP)[:, :, 1 : 1 + W]
        out[b, :, qh * HS : (qh + 1) * HS, :] = o
    return out


# revision 46
# speedup vs baseline: 1.1084x; 1.0125x over previous
"""Trainium2 Bass kernel for nn_LocalAttn: grouped local attention (3x3 window).

Sharding: 8 cores = batch(2) x H-strips(4). Each core gets a 34-row slice
(32 output rows + 1 halo row each side) of the W-and-H zero-padded input,
so all cores run one identical SPMD program.

v2 design (channel-major, pixels on the free dim, W padded to 130):
  conv1 (PE, bf16 block-diag, x in bf16) -> BN1+bias+tanh (ScalarE) -> t
  (f32r). conv2 is FUSED into a direct 3x3 logits conv on the PE:
  logits[8k+g] = w2m'.t(center) + w2n'.t(+dk), PSUM-accumulated over the
  9 offsets with BN2 scales folded into the weights. Boundary cells of
  t's group-0 rows are set to u = solve(W2n_s^T u = -c2n) so the fused
  neighbor term exactly cancels the folded BN bias where the reference
  zero-pads. exp via ScalarE (bias = folded BN2 offset) -> e72 bf16;
  softmax denom via ones-matmul (PE) -> reciprocal (DVE) -> 1/denom
  broadcast over k via sel-matmul (PE) -> e72 normalized in place (DVE).
  Normalized e72 is staged to DRAM per fifth so broadcast DMAs (stride-0
  DRAM source dims) expand 8 group rows -> 128 channel rows straight
  into SBUF bf16 at pure DMA-queue cost. Apply, per (fifth, quad): 9
  broadcast DMAs on SP/ACT queues feed 9 multiplies (6 on DVE at bf16
  2x rate, 3 on Pool tensor_tensor), bf16 add tree on DVE, last add on
  Pool -> f32 out tile -> DMA out. Value conv (PE, bf16) -> ScalarE
  copies to bf16 v, interleaved with the conv phases.
"""

import numpy as np
import ml_dtypes

import concourse.bass as bass
import concourse.bacc as bacc
import concourse.mybir as mybir
from concourse import tile
from concourse.bass_utils import run_bass_kernel_spmd

F32 = mybir.dt.float32
F32R = mybir.dt.float32r
BF16 = mybir.dt.bfloat16
AF = mybir.ActivationFunctionType
ALU = mybir.AluOpType

EPS = 1e-5
G = 8          # groups
B = 2
C = 256
H = W = 128
HS = 32        # output rows per core
HI = 34        # input rows per core (with halo)
WP = 130       # padded width
NIN = HI * WP          # 4420
NOUT = HS * WP         # 4160
NPAD = NIN + 2         # t / v free size, data at base offset 1

# phase A/C row-chunking (PSUM free dim <= 512 f32)
ACH = 3                # rows per conv chunk
# apply stages (pixel offset, size): small first stage unlocks the apply
# as soon as the first conv2 chunk is done; small last stage shortens the
# drain. Rows per stage: 2, 6, 10, 10, 4.
STAGES = [(0, 260), (260, 780), (1040, 1300), (2340, 1300), (3640, 520)]
SMAX = 1300

# free-dim offset into a base-1 padded [.., NPAD] tensor for the (dy,dx)
# neighbor of output pixel 0 (= input row 1, col 0)
def _koff(k):
    dy, dx = k // 3 - 1, k % 3 - 1
    return 1 + WP + dy * WP + dx


# apply-phase mul ownership: which k's multiply on DVE vs Pool
KS_DVE = (0, 1, 2, 3, 4, 5)   # bf16 SBUF muls on DVE
KS_POOLS = (6, 7, 8)          # bf16 SBUF muls on Pool (tensor_tensor)

_NC_CACHE = {}


def _build_nc():
    nc = bacc.Bacc("TRN2", target_bir_lowering=False, debug=False, num_devices=8)

    x_d = nc.dram_tensor("x", [2, 128, NIN], BF16, kind="ExternalInput")
    ca_d = nc.dram_tensor("cpkA", [128, 320], BF16, kind="ExternalInput")
    cs_d = nc.dram_tensor("cpkS", [64, 8], F32, kind="ExternalInput")
    cf2_d = nc.dram_tensor("cpkF", [72, 721], F32R, kind="ExternalInput")
    cb_d = nc.dram_tensor("cpkB", [72, 80], BF16, kind="ExternalInput")
    out_d = nc.dram_tensor("out", [2, 128, NOUT], F32, kind="ExternalOutput")
    # DRAM scratch for broadcast staging
    e72_d = nc.dram_tensor("e72d", [72, NOUT], BF16, kind="Internal")

    nch = HI // ACH + (1 if HI % ACH else 0)       # 12 input chunks (11x3+1)
    ncho = HS // ACH + (1 if HS % ACH else 0)      # 11 output chunks (10x3+2)

    with tile.TileContext(nc) as tc:
        with (
            nc.allow_low_precision(reason="bf16 softmax weights/values"),
            tc.tile_pool(name="const", bufs=1) as cp,
            tc.tile_pool(name="mid", bufs=1) as mp,
        ):
            # ---- input loads first (SP/ACT), then weights ----
            xq = []
            for q in range(2):
                xt = mp.tile([128, NIN], BF16, tag=f"x_{q}", name=f"x_{q}")
                xq.append(xt)
            xrows = [(0, 4), (4, 9), (9, 16), (16, 25), (25, 34)]
            for ci, (ra, rb_) in enumerate(xrows):
                fsl = slice(ra * WP, rb_ * WP)
                for q in range(2):
                    eng = nc.sync if (ci + q) % 2 == 0 else nc.scalar
                    eng.dma_start(xq[q][:, fsl], x_d[q, :, fsl])

            # ---- constant / weight loads (packed, few DMAs) ----
            cA = cp.tile([128, 320], BF16, tag="cA", name="cA")
            nc.gpsimd.dma_start(cA[:], ca_d[:])
            w1t = [cA[:, 0:32], cA[:, 32:64]]
            wvt = [cA[:, 64:192], cA[:, 192:320]]
            cS = cp.tile([64, 8], F32, tag="cS", name="cS")
            nc.gpsimd.dma_start(cS[:], cs_d[:])
            s1t = cS[:, 0:1]
            c1t = cS[:, 1:2]
            atop = cS[0:8, 2:3]
            btop = cS[0:8, 3:4]
            abot = cS[0:8, 4:5]
            bbot = cS[0:8, 5:6]
            ubc = cS[0:8, 6:7]
            cF = cp.tile([72, 721], F32R, tag="cF", name="cF")
            nc.gpsimd.dma_start(cF[:], cf2_d[:])
            w2mt = cF[0:64, 0:72]
            w2nt = [cF[0:8, 72 + 72 * k : 144 + 72 * k] for k in range(9)]
            cft = cF[0:72, 720:721]
            cB = cp.tile([72, 80], BF16, tag="cB", name="cB")
            nc.gpsimd.dma_start(cB[:], cb_d[:])
            onest = cB[0:72, 0:8]
            r72t = cB[0:8, 8:80]

            # ---- persistent mid tensors ----
            t_sb = mp.tile([64, NPAD], F32R, tag="t", name="t")
            v_sb = [mp.tile([128, NPAD], BF16, tag=f"v_{q}", name=f"v_{q}") for q in range(2)]
            e72 = mp.tile([72, NOUT], BF16, tag="e72", name="e72")
            rb = mp.tile([8, NOUT], BF16, tag="rb", name="rb")

            # ---- phases A+C interleaved: conv1/tanh/vconv then fused
            # conv2/exp/denom two chunks behind, sharing one PSUM scope ----
            for q in range(2):
                nc.gpsimd.memset(v_sb[q][:, 0:1], 0.0)
                nc.gpsimd.memset(v_sb[q][:, NPAD - 1 : NPAD], 0.0)
            with (
                tc.tile_pool(name="pc64", bufs=2, space="PSUM") as pc64,
                tc.tile_pool(name="pv", bufs=2, space="PSUM") as pvp,
                tc.tile_pool(name="pcm", bufs=2, space="PSUM") as pcm,
                tc.tile_pool(name="pcd", bufs=1, space="PSUM") as pcd,
                tc.tile_pool(name="pcr", bufs=1, space="PSUM") as pcr,
            ):
                def a_chunk(ch):
                    r0 = ch * ACH
                    r1 = min(r0 + ACH, HI)
                    f0 = r0 * WP
                    fsz = (r1 - r0) * WP
                    sl = slice(f0, f0 + fsz)
                    pt = pc64.tile([64, fsz], F32, tag="pc", name="pc", padded_shape=[64, 512])
                    nc.tensor.matmul(
                        pt[0:32, :], w1t[0],
                        xq[0][:, sl],
                        start=True, stop=True, tile_position=(0, 0),
                    )
                    nc.tensor.matmul(
                        pt[32:64, :], w1t[1],
                        xq[1][:, sl],
                        start=True, stop=True, tile_position=(0, 32),
                    )
                    nc.scalar.activation(
                        t_sb[:, 1 + f0 : 1 + f0 + fsz], pt[:],
                        AF.Tanh, bias=c1t, scale=s1t,
                    )
                    # boundary cells of t (group-0 rows): halo rows become
                    # t*a + b (a,b host-set: u at image boundary, identity
                    # elsewhere); W-pad columns always become u, so the fused
                    # neighbor term cancels the folded BN bias off-image
                    if ch == 0:
                        nc.vector.tensor_scalar(
                            t_sb[0:8, 1 : 1 + WP], t_sb[0:8, 1 : 1 + WP],
                            atop, btop, ALU.mult, ALU.add,
                        )
                    if ch == nch - 1:
                        nc.vector.tensor_scalar(
                            t_sb[0:8, 1 + 33 * WP : 1 + 34 * WP],
                            t_sb[0:8, 1 + 33 * WP : 1 + 34 * WP],
                            abot, bbot, ALU.mult, ALU.add,
                        )
                    nc.vector.tensor_scalar(
                        t_sb[0:8, f0 : f0 + fsz : WP],
                        xq[0][0:8, 0 : fsz : WP],
                        0.0, ubc, ALU.mult, ALU.add,
                    )
                    nc.vector.tensor_scalar(
                        t_sb[0:8, 1 + f0 : 1 + f0 + fsz : WP],
                        xq[0][0:8, 0 : fsz : WP],
                        0.0, ubc, ALU.mult, ALU.add,
                    )
                    if ch == nch - 1:
                        nc.vector.tensor_scalar(
                            t_sb[0:8, f0 + fsz : NPAD],
                            xq[0][0:8, 0 : NPAD - f0 - fsz],
                            0.0, ubc, ALU.mult, ALU.add,
                        )
                def v_chunk(ch):
                    r0 = ch * ACH
                    r1 = min(r0 + ACH, HI)
                    f0 = r0 * WP
                    fsz = (r1 - r0) * WP
                    sl = slice(f0, f0 + fsz)
                    for q in range(2):
                        pv = pvp.tile([128, fsz], F32, tag="pv", name="pv", padded_shape=[128, 512])
                        nc.tensor.matmul(
                            pv[:], wvt[q],
                            xq[q][:, sl],
                            start=True, stop=True,
                        )
                        # early chunks: copy on DVE (idle during the ramp),
                        # late chunks: ACT (DVE saturated by then)
                        if ch < 3:
                            nc.vector.tensor_copy(
                                v_sb[q][:, 1 + f0 : 1 + f0 + fsz], pv[:]
                            )
                        else:
                            nc.scalar.copy(v_sb[q][:, 1 + f0 : 1 + f0 + fsz], pv[:])

                def c_chunk(ch):
                    r0 = ch * ACH
                    r1 = min(r0 + ACH, HS)
                    fsz = (r1 - r0) * WP
                    o0 = r0 * WP                     # offset in out space
                    tbase = 1 + WP + o0              # center in t space
                    pm = pcm.tile([72, fsz], F32, tag="pm", name="pm", padded_shape=[72, 512])
                    nc.tensor.matmul(
                        pm[:], w2mt,
                        t_sb[:, tbase : tbase + fsz],
                        start=True, stop=False, skip_group_check=True,
                    )
                    for k in range(9):
                        dy, dx = k // 3 - 1, k % 3 - 1
                        tb = tbase + dy * WP + dx
                        nc.tensor.matmul(
                            pm[:],
                            w2nt[k],
                            t_sb[0:8, tb : tb + fsz],
                            start=False, stop=(k == 8), skip_group_check=True,
                        )
                    nc.scalar.activation(
                        e72[:, o0 : o0 + fsz], pm[:],
                        AF.Exp, bias=cft,
                    )
                    pd = pcd.tile([8, fsz], F32, tag="pd", name="pd", padded_shape=[8, 512])
                    nc.tensor.matmul(
                        pd[:], onest, e72[:, o0 : o0 + fsz],
                        start=True, stop=True,
                    )
                    nc.vector.reciprocal(rb[:, o0 : o0 + fsz], pd[:])
                    pr72 = pcr.tile([72, fsz], F32, tag="pr72", name="pr72", padded_shape=[72, 512])
                    nc.tensor.matmul(
                        pr72[:], r72t, rb[:, o0 : o0 + fsz],
                        start=True, stop=True,
                    )
                    nc.vector.tensor_mul(
                        e72[:, o0 : o0 + fsz], e72[:, o0 : o0 + fsz], pr72[:]
                    )

                LAG = 2
                VLAG = 4
                for ch in range(nch + ncho):
                    if ch < nch:
                        a_chunk(ch)
                    if LAG <= ch and ch - LAG < ncho:
                        c_chunk(ch - LAG)
                    if VLAG <= ch and ch - VLAG < nch:
                        v_chunk(ch - VLAG)
                for ch in range(nch - VLAG, nch):
                    v_chunk(ch)

            # stage normalized e72 to DRAM (for the pab broadcasts)
            for h0, sz in STAGES:
                nc.sync.dma_start(e72_d[:, h0 : h0 + sz], e72[:, h0 : h0 + sz])

            # ---- phase D: apply ----
            with (
                nc.allow_low_precision(reason="3x3 softmax-weighted sum in bf16"),
                tc.tile_pool(name="pab", bufs=14) as pabp,
                tc.tile_pool(name="prod", bufs=11) as prp,
                tc.tile_pool(name="sums", bufs=6) as smp,
                tc.tile_pool(name="outp", bufs=3) as outp,
            ):
                for h, (h0, sz) in enumerate(STAGES):
                    for q in range(2):
                        hsl = slice(h0, h0 + sz)
                        pab = {}
                        for k in range(9):
                            pt = pabp.tile([128, SMAX], BF16, tag="pab", name="pab")
                            src_ap = (
                                e72_d[8 * k + 4 * q : 8 * k + 4 * q + 4, hsl]
                                .unsqueeze(1).broadcast_to([4, 32, sz])
                            )
                            eng = nc.sync if k in (0, 2, 4, 6, 7, 8) else nc.scalar
                            eng.dma_start(pt[:, 0:sz], src_ap)
                            pab[k] = pt
                        prod = {}
                        for k in range(9):
                            voff = _koff(k) + h0
                            vsl = v_sb[q][:, voff : voff + sz]
                            pr = prp.tile([128, SMAX], BF16, tag="pr", name="pr")
                            if k in KS_DVE and not (k == 5 and q == 0):
                                nc.vector.tensor_mul(pr[:, 0:sz], pab[k][:, 0:sz], vsl)
                            else:
                                nc.gpsimd.tensor_mul(pr[:, 0:sz], pab[k][:, 0:sz], vsl)
                            prod[k] = pr
                        # balanced bf16 add tree on DVE; final add on Pool
                        s01 = smp.tile([128, SMAX], BF16, tag="s", name="s01")
                        nc.vector.tensor_add(s01[:, 0:sz], prod[0][:, 0:sz], prod[1][:, 0:sz])
                        s23 = smp.tile([128, SMAX], BF16, tag="s", name="s23")
                        nc.vector.tensor_add(s23[:, 0:sz], prod[2][:, 0:sz], prod[3][:, 0:sz])
                        s45 = smp.tile([128, SMAX], BF16, tag="s", name="s45")
                        nc.vector.tensor_add(s45[:, 0:sz], prod[4][:, 0:sz], prod[5][:, 0:sz])
                        nc.vector.tensor_add(s01[:, 0:sz], s01[:, 0:sz], s23[:, 0:sz])
                        nc.vector.tensor_add(s01[:, 0:sz], s01[:, 0:sz], s45[:, 0:sz])
                        nc.vector.tensor_add(s01[:, 0:sz], s01[:, 0:sz], prod[6][:, 0:sz])
                        nc.vector.tensor_add(s01[:, 0:sz], s01[:, 0:sz], prod[7][:, 0:sz])
                        ot = outp.tile([128, SMAX], F32, tag="ot", name="ot")
                        nc.gpsimd.tensor_add(ot[:, 0:sz], s01[:, 0:sz], prod[8][:, 0:sz])
                        nc.gpsimd.dma_start(out_d[q, :, hsl], ot[:, 0:sz])

    nc.compile()
    return nc


def _host_prep(x, w1, b1, g1, be1, m1, v1, w2, b2, g2, be2, m2, v2, wv):
    f32 = np.float32

    inv1 = (g1 / np.sqrt(v1 + EPS)).astype(f32)            # [64]
    s1 = inv1
    c1 = (b1 * inv1 + be1 - m1 * inv1).astype(f32)
    inv2 = (g2 / np.sqrt(v2 + EPS)).astype(f32)            # [80]
    s2r = inv2
    c2r = (b2 * inv2 + be2 - m2 * inv2).astype(f32)

    # conv2 output layout: psum row j = 8k+g -> ref mask ch 8+9g+k;
    # neighbor path: ref ch g (g<8), i.e. group 0 of t, co=g
    mperm = np.zeros(72, dtype=np.int64)
    for k in range(9):
        for g in range(8):
            mperm[8 * k + g] = 8 + 9 * g + k
    s2m = s2r[mperm]
    c2m = c2r[mperm]
    s2n = s2r[:8]
    c2n = c2r[:8]

    # conv1 block-diag lhsT per quad: [128, 32]
    w1bd = np.zeros((2, 128, 32), dtype=f32)
    for q in range(2):
        for gh in range(4):
            g = 4 * q + gh
            w1bd[q, 32 * gh : 32 * gh + 32, 8 * gh : 8 * gh + 8] = w1[g].T

    # fused conv2 weights with BN2 scales folded in
    w2m = np.zeros((64, 72), dtype=f32)
    for j in range(72):
        r = mperm[j]
        gc, co = r // 10, r % 10
        w2m[8 * gc : 8 * gc + 8, j] = w2[gc, co, :] * s2m[j]
    w2n = np.zeros((9, 8, 72), dtype=f32)
    for k in range(9):
        for g in range(8):
            gc, co = g // 10, g % 10      # ref ch g -> group 0, co g
            w2n[k, :, 8 * k + g] = w2[gc, co, :] * s2n[g]
    cf = (c2m + c2n[np.arange(72) % 8]).astype(f32)
    # boundary vector u: W_s^T u = -c2n, with W_s[ci, g] = w2[0, g, ci]*s2n[g]
    Ws = (w2[0, 0:8, :].T * s2n[None, :]).astype(np.float64)   # [ci, g]
    ubc = np.linalg.solve(Ws.T, -c2n.astype(np.float64)).astype(f32)

    # value conv block-diag lhsT per quad: [128, 128]
    wvbd = np.zeros((2, 128, 128), dtype=f32)
    for q in range(2):
        for gh in range(4):
            g = 4 * q + gh
            wvbd[q, 32 * gh : 32 * gh + 32, 32 * gh : 32 * gh + 32] = wv[g].T

    onesb = np.zeros((72, 8), dtype=ml_dtypes.bfloat16)
    for k in range(9):
        for g in range(8):
            onesb[8 * k + g, g] = 1
    rsel72 = np.zeros((8, 72), dtype=ml_dtypes.bfloat16)
    for k in range(9):
        for g in range(8):
            rsel72[g, 8 * k + g] = 1

    # packed const blocks
    cpkA = np.zeros((128, 320), dtype=ml_dtypes.bfloat16)
    cpkA[:, 0:32] = w1bd[0]
    cpkA[:, 32:64] = w1bd[1]
    cpkA[:, 64:192] = wvbd[0]
    cpkA[:, 192:320] = wvbd[1]
    cpkF = np.zeros((72, 721), dtype=f32)
    cpkF[0:64, 0:72] = w2m
    for k in range(9):
        cpkF[0:8, 72 + 72 * k : 144 + 72 * k] = w2n[k]
    cpkF[0:72, 720] = cf
    cpkB = np.zeros((72, 80), dtype=ml_dtypes.bfloat16)
    cpkB[0:72, 0:8] = onesb
    cpkB[0:8, 8:80] = rsel72

    # padded input: (2, 256, 130, 130)
    xp = np.zeros((B, C, H + 2, W + 2), dtype=f32)
    xp[:, :, 1:-1, 1:-1] = x

    shards = []
    for b in range(B):
        for qh in range(4):
            xs = xp[b, :, qh * HS : qh * HS + HI, :]       # [256, 34, 130]
            xs = np.ascontiguousarray(
                xs.reshape(2, 128, NIN).astype(ml_dtypes.bfloat16)
            )
            cpkS = np.zeros((64, 8), dtype=f32)
            cpkS[:, 0] = s1
            cpkS[:, 1] = c1
            if qh == 0:
                cpkS[0:8, 2] = 0.0
                cpkS[0:8, 3] = ubc
            else:
                cpkS[0:8, 2] = 1.0
                cpkS[0:8, 3] = 0.0
            if qh == 3:
                cpkS[0:8, 4] = 0.0
                cpkS[0:8, 5] = ubc
            else:
                cpkS[0:8, 4] = 1.0
                cpkS[0:8, 5] = 0.0
            cpkS[0:8, 6] = ubc
            shards.append(
                {
                    "x": xs,
                    "cpkA": cpkA, "cpkS": cpkS, "cpkF": cpkF, "cpkB": cpkB,
                }
            )
    return shards


def kernel(**inputs):
    if "nc" not in _NC_CACHE:
        _NC_CACHE["nc"] = _build_nc()
    nc = _NC_CACHE["nc"]

    shards = _host_prep(**inputs)
    res = run_bass_kernel_spmd(nc, shards, core_ids=list(range(8)))

    out = np.zeros((B, C, H, W), dtype=np.float32)
    for i, r in enumerate(res.results):
        b, qh = divmod(i, 4)
        o = r["out"].reshape(C, HS, WP)[:, :, 1 : 1 + W]
        out[b, :, qh * HS : (qh + 1) * HS, :] = o
    return out
